# revision 46
# baseline (speedup 1.0000x reference)
"""MoE routing transformer block on 8 trn2 NeuronCores.

Strategy: the reference's (top-k slot kk, expert e) pairs partition the
T=2048 tokens into 8 independent groups (2 slots x 4 experts), each running a
full pre-LN attention+MLP block with attention restricted to the group.
One NeuronCore per (kk, e) pair.

Host: computes the (tiny) router gate + top-2 routing in numpy, gathers each
group's tokens, computes LN1, pre-packs weights into [128, ...] partition
layouts (fp8e4, x16-scaled), launches one SPMD bass kernel on the 8 cores,
then scatter-adds the gate-prob-weighted outputs back.

Device (per core, [feature, token] layout, C = padded group capacity):
  qkT = (WqkT.T @ hT) * (1/128 | 1/16) + bqk   fp8 DoubleRow, bf16 out
  v   = hT.T @ WvT                             fp8 DoubleRow, fp8 out (16v),
                                               col 64 of 66/head = 16.0 (den)
  sT[k,q] = kT_h.T @ qT_h                      bf16, per (head, key-tile)
  expT = exp(sT + key_bias) -> fp8             key_bias kills padded keys
  po[d,q], den[q] = v_aug.T @ expT             fp8 DoubleRow over key pairs
  rden = exp(-ln(den))                         2 ACT ops for all 8 heads
  onorm = po * bcast(rden)                     fp8 out
  x1T = (WoT.T @ onorm)/16 + (xT + bo)         fp8 DoubleRow + one DVE op
  h2T = LN2(x1T) -> fp8                        stats via ones-matmul
  gT = gelu((W1T.T @ h2T)/16 + b1) -> fp8      fp8 DoubleRow, ACT scale fold
  yT = (W2T.T @ gT)/16 + b2 + x1T              fp8 DoubleRow + one DVE op
"""

import os
import numpy as np
import ml_dtypes

import concourse.bass as bass
import concourse.mybir as mybir
import concourse.tile as tile
import concourse.tile_utils as tile_utils
from concourse import bass_utils


def _install_ntff_shim():
    """This image's antenv lacks axon_hooks; synthesize it so trace=True works."""
    import sys as _sys
    import types as _types
    try:
        import antenv.axon_hooks  # noqa: F401
        return
    except ImportError:
        pass
    try:
        from trn_agent_boot.trn_boot import _ntff_profile_via_ctypes
        hook = _ntff_profile_via_ctypes('/opt/axon/libaxon_pjrt.so')
    except Exception:
        hook = None
    mod = _types.ModuleType('antenv.axon_hooks')
    state = {'hook': hook}
    mod.set_axon_ntff_profile_hook = lambda h: state.__setitem__('hook', h)
    mod.get_axon_ntff_profile_hook = lambda: state['hook']
    _sys.modules['antenv.axon_hooks'] = mod
    try:
        import antenv
        antenv.axon_hooks = mod
    except ImportError:
        pass


_install_ntff_shim()

# stale constant leaves 16KiB/partition unused on trn2 (224 phys / 208 usable)
tile_utils.max_sbuf_usage = 208 * 1024

E = 512
H = 8
D = 64
HID = 2048
NE = 4
TOPK = 2
EPS = 1e-5

f32 = mybir.dt.float32
f32r = mybir.dt.float32r
bf16 = mybir.dt.bfloat16
f8 = mybir.dt.float8e4
AF = mybir.ActivationFunctionType
ALU = mybir.AluOpType
DR = mybir.MatmulPerfMode.DoubleRow
np8 = ml_dtypes.float8_e4m3

KEY_PAD_BIAS = -60.0
WS = 16.0          # fp8 weight pre-scale


# ---------------------------------------------------------------------------
# walrus in this container encodes at most one sync wait per instruction;
# Tile's kernel-tail drain can carry several. Split extras onto NoOps.
def _split_excess_waits(nc):
    for fn in nc.m.functions:
        for blk in fn.blocks:
            new_insts = []
            for ins in blk.instructions:
                si = ins.sync_info
                if si is not None and len(si.on_wait) > 1:
                    waits = list(si.on_wait)
                    excess, keep = waits[:-1], waits[-1:]
                    for w in excess:
                        new_insts.append(mybir.InstNoOp(
                            name=f"I-waitsplit-{nc.next_id()}",
                            engine=ins.engine, ins=[], outs=[],
                            sync_info=mybir.SyncInfo(on_wait=[w], on_update=[]),
                        ))
                    si.on_wait = keep
                new_insts.append(ins)
            blk.instructions[:] = new_insts


def _build(C, has_vbias=False, has_b2=False):
    """Build the bass program for group capacity C (multiple of 64)."""
    assert C % 64 == 0
    KT = -(-C // 128)                       # key/token tiles (last may be 64)
    kts = [(i * 128, min(128, C - i * 128)) for i in range(KT)]
    nfull = sum(1 for _, s in kts if s == 128)
    npair = nfull // 2                      # DoubleRow AV pairs
    tails = list(range(2 * npair, KT))      # plain-fp8 AV tiles
    if C <= 512:
        NCH, CSZ = 1, C
    else:
        NCH, CSZ = 2, C // 2
        assert CSZ <= 512
    CH = [(i * CSZ, CSZ) for i in range(NCH)]

    nc = bass.Bass(num_swdge_queues=4)

    NCONST = KT + 8 + 16 + 4 + 4
    consts_d = nc.dram_tensor("consts", [128, NCONST], f32, kind="ExternalInput")
    # f32r constants must come via DMA: memset on f32r fails walrus' ISA check
    selp_d = nc.dram_tensor("selp", [128, 385], f32r, kind="ExternalInput")
    hT_d = nc.dram_tensor("hT", [128, 4 * C], f8, kind="ExternalInput")
    wqk_d = nc.dram_tensor("wqk", [128, 4 * 1024], f8, kind="ExternalInput")
    wv_d = nc.dram_tensor("wv", [128, 4 * 512], f8, kind="ExternalInput")
    wo_d = nc.dram_tensor("wo", [128, 4 * 512], f8, kind="ExternalInput")
    w1_d = nc.dram_tensor("w1", [128, 4 * 2048], f8, kind="ExternalInput")
    w2_d = nc.dram_tensor("w2", [128, 16 * 512], f8, kind="ExternalInput")
    xTb_d = nc.dram_tensor("xTb", [128, 4 * C], f32, kind="ExternalInput")
    if has_vbias:
        wvb_d = nc.dram_tensor("wvb", [1, 512], f8, kind="ExternalInput")
    out_d = nc.dram_tensor("yT", [128, 4 * C], bf16, kind="ExternalOutput")

    def dr4(d, t):
        return d[:].rearrange("p (t c) -> p t c", t=t)

    with tile.TileContext(nc) as tc, nc.allow_low_precision(
            reason="fp8/bf16 rounding on matmul-feeding tiles is intended"):
        with (
            tc.tile_pool(name="const", bufs=1) as cpool,
            tc.tile_pool(name="main", bufs=1) as mpool,
            tc.tile_pool(name="expp", bufs=3) as expp,
            tc.tile_pool(name="scr", bufs=2) as scr,
        ):
            # ---- const tiles (DMAs deferred past the hot path) ----
            cst = cpool.tile([128, NCONST], f32)
            o = [0]
            def _csl(n):
                a = o[0]; o[0] += n
                return cst[:, a:a + n]
            kb, bqk, b1, l2w, b2c = _csl(KT), _csl(8), _csl(16), _csl(4), _csl(4)
            selAB = cpool.tile([128, 2, 128], f32r)
            ones_row = cpool.tile([1, 128], f32r)
            ecolr = cpool.tile([128, 1], f32r)
            ecolb = cpool.tile([128, 1], bf16)
            eps_t = cpool.tile([1, 1], f32)
            nc.vector.memset(eps_t[:], EPS)
            if has_vbias:
                onesb8 = cpool.tile([1, 128], f8)
                nc.vector.memset(onesb8[:], 1.0)

            # ---- big tiles; DMAs in need-order, partition-split for ----
            # ---- queue parallelism                                    ----
            hT = mpool.tile([128, 4, C], f8, tag="hT")
            wqk = mpool.tile([128, 4, 1024], f8, tag="wqk")
            wv = mpool.tile([128, 4, 512], f8, tag="wv")
            wo = mpool.tile([128, 4, 512], f8, tag="wo")
            w1 = mpool.tile([128, 4, 2048], f8, tag="w1")
            w2 = mpool.tile([128, 16, 512], f8, tag="w2")
            xTb = mpool.tile([128, 4, C], f32, tag="xTb")
            qkT = mpool.tile([128, 8, C], bf16, tag="qkT")
            v = mpool.tile([128, KT, 8 * 66], f8, tag="v")
            onormU = mpool.tile([128, 4, C], bf16, tag="onU")
            onormN = mpool.tile([128, 4, C], f8, tag="onN")
            x1T = mpool.tile([128, 4, C], f32r, tag="x1T")
            sq = mpool.tile([128, 4, C], bf16, tag="sq")
            gT = mpool.tile([128, 16, C], f8, tag="gT")
            yT = mpool.tile([128, 4, C], bf16, tag="yT")
            # denominators: head h at partition 32*(h%4), free slot h//4.
            # 1/den = exp(-ln(den)) batched on ACT (2 instrs for all heads);
            # partition bases must be 32-aligned on every engine.
            rden = mpool.tile([128, 2, NCH, CSZ], f32, tag="rden")
            lnden = mpool.tile([128, 2, NCH, CSZ], f32, tag="lnden")
            denr = mpool.tile([128, 2, NCH, CSZ], f32r, tag="denr")
            if has_b2:
                x1b = mpool.tile([128, 4, C], f32r, tag="x1b")
            else:
                x1b = x1T

            def dma_split(t, d, tdim, nsplit, spread=False):
                dv = dr4(d, tdim)
                engs = ((nc.sync, nc.scalar, nc.gpsimd)
                        if spread else (nc.sync,))
                P = 128 // nsplit
                for i in range(nsplit):
                    engs[i % len(engs)].dma_start(t[P * i:P * (i + 1)],
                                                  dv[P * i:P * (i + 1)])

            dma_split(hT, hT_d, 4, 4, spread=True)
            dma_split(wv, wv_d, 4, 2, spread=True)
            dma_split(wqk, wqk_d, 4, 4, spread=True)
            nc.sync.dma_start(cst[:], consts_d[:])
            if has_vbias:
                wvb = cpool.tile([1, 512], f8)
                nc.sync.dma_start(wvb[:], wvb_d[:])
            dma_split(wo, wo_d, 4, 2)
            dma_split(w1, w1_d, 4, 4)
            nc.sync.dma_start(selAB[:], selp_d[:, 0:256].rearrange(
                "p (t c) -> p t c", t=2))
            nc.sync.dma_start(ones_row[:], selp_d[0:1, 256:384])
            nc.sync.dma_start(ecolr[:], selp_d[:, 384:385])
            nc.vector.tensor_copy(ecolb[:], ecolr[:])
            dma_split(w2, w2_d, 16, 4)
            dma_split(xTb, xTb_d, 4, 4)

            # flat init: col 64 of each 66-wide head group is the
            # denominator marker (16.0); data cols 0-63 overwritten by the
            # v-proj copies; col 65 is alignment padding (fp8 slices need
            # even byte offsets)
            # unwritten rden rows must be finite (0 * NaN poisons the
            # broadcast matmul); ln(1)=0 -> exp(0)=1
            nc.gpsimd.memset(rden[:], 1.0)

            nc.gpsimd.memset(v[:], WS)

            def pview(p):
                if NCH == 1:
                    return p[:, 0, 0:C]
                return p[:, :, 0:CSZ]

            psA_cm = tc.tile_pool(name="psA", bufs=2, space="PSUM")
            ps = psA_cm.__enter__()
            pso_cm = tc.tile_pool(name="pso", bufs=2, space="PSUM")
            ps_o = pso_cm.__enter__()

            # ---- v (normal layout, heads in 65-col groups, 16x scaled) ----
            for tt in range(KT):
                toff, tsz = kts[tt]
                p = ps.tile([128, 2, 512], f32, tag="b2", name=f"v{tt}")
                for i in range(2):
                    nc.tensor.matmul(p[0:tsz, 0, :],
                                     hT[:, 2 * i:2 * i + 2, toff:toff + tsz],
                                     wv[:, 2 * i:2 * i + 2, :],
                                     start=(i == 0),
                                     stop=(i == 1 and not has_vbias),
                                     perf_mode=DR)
                if has_vbias:
                    nc.tensor.matmul(p[0:tsz, 0, :], onesb8[0:1, 0:tsz],
                                     wvb[0:1, :], start=False, stop=True)
                nc.vector.tensor_copy(
                    v[0:tsz, tt, :].rearrange("p (h x) -> p h x", x=66)[:, :, 0:64],
                    p[0:tsz, 0, :].rearrange("p (h x) -> p h x", x=64))

            # ---- qkT: k slots first so attention can start early ----
            for nt in (4, 0, 5, 1, 6, 2, 7, 3):
                p = ps.tile([128, 2, 512], f32, tag="b2", name=f"qk{nt}")
                for i in range(2):
                    for ci, (off, sz) in enumerate(CH):
                        nc.tensor.matmul(p[:, ci, 0:sz],
                                         wqk[:, 2 * i:2 * i + 2,
                                             128 * nt:128 * (nt + 1)],
                                         hT[:, 2 * i:2 * i + 2, off:off + sz],
                                         start=(i == 0), stop=(i == 1),
                                         perf_mode=DR)
                if nt < 4:      # q on ACT, k on DVE: conversions in parallel
                    nc.scalar.activation(qkT[:, nt, :], pview(p), AF.Identity,
                                         bias=bqk[:, nt:nt + 1],
                                         scale=1.0 / (WS * 8.0))
                else:
                    nc.vector.tensor_scalar(qkT[:, nt, :], pview(p), 1.0 / WS,
                                            bqk[:, nt:nt + 1],
                                            op0=ALU.mult, op1=ALU.add)

            # ---- attention ----
            def av_pair_f(h, po, et, i):
                for ci in range(NCH):
                    nc.tensor.matmul(po[0:66, ci, 0:CSZ],
                                     v[:, 2 * i:2 * i + 2,
                                       66 * h:66 * h + 66],
                                     et[:, 2 * i:2 * i + 2, ci, :],
                                     start=(i == 0),
                                     stop=(i == npair - 1 and not tails),
                                     perf_mode=DR)

            def attn_tail(h, po, et):
                # deferred: last AV pair, tail tiles, then drain po
                if npair >= 1:
                    av_pair_f(h, po, et, npair - 1)
                for tx, kt in enumerate(tails):
                    koff, ksz = kts[kt]
                    for ci in range(NCH):
                        nc.tensor.matmul(po[0:66, ci, 0:CSZ],
                                         v[0:ksz, kt, 66 * h:66 * h + 66],
                                         et[0:ksz, kt, ci, :],
                                         start=(npair == 0 and tx == 0),
                                         stop=(tx == len(tails) - 1))
                bp = 64 * (h % 2)
                nc.vector.tensor_copy(onormU[bp:bp + 64, h // 2, :],
                                      pview(po)[0:64])
                dp = 32 * (h % 4)
                nc.vector.tensor_copy(
                    rden[dp:dp + 1, h // 4, :, :],
                    po[64:65, :, 0:CSZ] if NCH == 2 else po[64:65, 0:1, 0:C])

            pend = [None]
            for h in range(H):
                bp = 64 * (h % 2)
                j = h // 2
                qT_h = qkT[bp:bp + 64, j, :]
                kT_h = qkT[bp:bp + 64, 4 + j, :]
                et = expp.tile([128, KT, NCH, CSZ], f8, tag="et", name=f"et{h}")
                po = ps_o.tile([66, 2, 512], f32, tag="po", name=f"po{h}")
                for kt in range(KT):
                    koff, ksz = kts[kt]
                    pss = ps.tile([128, 2, 512], f32, tag="b2",
                                  name=f"s{h}_{kt}")
                    for ci, (off, sz) in enumerate(CH):
                        nc.tensor.matmul(pss[0:ksz, ci, 0:sz],
                                         kT_h[:, koff:koff + ksz],
                                         qT_h[:, off:off + sz],
                                         start=True, stop=True)
                    if kt == 0 and pend[0] is not None:
                        # prev head's AV tail runs behind this head's QK0,
                        # so exp(h,0) is never gated by it
                        attn_tail(*pend[0])
                        pend[0] = None
                    if kt >= 2 and kt % 2 == 0 and (kt - 2) // 2 < npair - 1:
                        av_pair_f(h, po, et, (kt - 2) // 2)
                    nc.scalar.activation(et[0:ksz, kt, :, :], pview(pss)[0:ksz],
                                         AF.Exp, bias=kb[0:ksz, kt:kt + 1])
                for i in range(max(0, (KT - 1) // 2), npair - 1):
                    av_pair_f(h, po, et, i)
                pend[0] = (h, po, et)
            attn_tail(*pend[0])

            # ---- normalize: bcast 1/den over 64 partitions per head ----
            nc.scalar.activation(lnden[:], rden[:], AF.Ln)
            nc.scalar.activation(denr[:], lnden[:], AF.Exp, scale=-1.0)
            for t in range(4):
                rp = ps.tile([128, 2, 512], f32, tag="b2", name=f"rp{t}")
                for ci in range(NCH):
                    nc.tensor.matmul(rp[:, ci, 0:CSZ], selAB[:, t % 2, :],
                                     denr[:, t // 2, ci, :], start=True,
                                     stop=True)
                nc.vector.tensor_mul(onormN[:, t, :], onormU[:, t, :],
                                     pview(rp))

            # ---- out proj + residual, LN2 stats fused in ----
            stm = ps_o.tile([1, 2, 512], f32, tag="po", name="stm")
            stq = ps_o.tile([1, 2, 512], f32, tag="po", name="stq")

            def ln2_stats(kt):
                nc.gpsimd.tensor_mul(sq[:, kt, :], x1T[:, kt, :], x1T[:, kt, :])
                for ci, (off, sz) in enumerate(CH):
                    nc.tensor.matmul(stm[0:1, ci, 0:sz], ecolr[:],
                                     x1T[:, kt, off:off + sz],
                                     start=(kt == 0), stop=(kt == 3))
                    nc.tensor.matmul(stq[0:1, ci, 0:sz], ecolb[:],
                                     sq[:, kt, off:off + sz],
                                     start=(kt == 0), stop=(kt == 3))

            for nt in range(4):
                p = ps.tile([128, 2, 512], f32, tag="b2", name=f"op{nt}")
                for i in range(2):
                    for ci, (off, sz) in enumerate(CH):
                        nc.tensor.matmul(p[:, ci, 0:sz],
                                         wo[:, 2 * i:2 * i + 2,
                                            128 * nt:128 * (nt + 1)],
                                         onormN[:, 2 * i:2 * i + 2, off:off + sz],
                                         start=(i == 0), stop=(i == 1),
                                         perf_mode=DR)
                nc.vector.scalar_tensor_tensor(
                    x1T[:, nt, :], pview(p), 1.0 / WS,
                    xTb[:, nt, :], op0=ALU.mult, op1=ALU.add)
                if nt >= 1:
                    ln2_stats(nt - 1)
            ln2_stats(3)
            def pv1(p):
                return p[0:1, :, 0:CSZ] if NCH == 2 else p[0:1, 0:1, 0:C]
            mu2 = scr.tile([1, C], f32, tag="lnt", name="mu2")
            nc.scalar.activation(mu2[0:1, :], pv1(stm), AF.Square)
            var = scr.tile([1, C], f32, tag="lnt", name="var")
            nc.vector.scalar_tensor_tensor(var[0:1, :], mu2[0:1, :], -1.0,
                                           pv1(stq), op0=ALU.mult, op1=ALU.add)
            lnv = scr.tile([1, C], f32, tag="lnt", name="lnv")
            nc.scalar.activation(lnv[:], var[:], AF.Ln, bias=eps_t[0:1, 0:1])
            rstd = scr.tile([1, C], f32r, tag="lnt", name="rstd")
            nc.scalar.activation(rstd[:], lnv[:], AF.Exp, scale=-0.5)
            mbneg = scr.tile([1, C], f32r, tag="lnt", name="mbneg")
            nc.vector.scalar_tensor_tensor(mbneg[:], pv1(stm), -1.0, rstd[:],
                                           op0=ALU.mult, op1=ALU.mult)
            pA = ps.tile([128, 2, 512], f32, tag="b2", name="lnA")
            pB = ps.tile([128, 2, 512], f32, tag="b2", name="lnB")
            for ci, (off, sz) in enumerate(CH):
                nc.tensor.matmul(pA[:, ci, 0:sz], ones_row[0:1, 0:128],
                                 rstd[0:1, off:off + sz], start=True, stop=True)
                nc.tensor.matmul(pB[:, ci, 0:sz], ones_row[0:1, 0:128],
                                 mbneg[0:1, off:off + sz], start=True, stop=True)
            for kt in range(4):
                tmp = scr.tile([128, C], f32, tag="lntmp", name=f"lt{kt}")
                nc.vector.scalar_tensor_tensor(
                    tmp[:], x1T[:, kt, :], l2w[:, kt:kt + 1], pview(pA),
                    op0=ALU.mult, op1=ALU.mult)
                nc.vector.scalar_tensor_tensor(
                    hT[:, kt, :], pview(pB), l2w[:, kt:kt + 1], tmp[:],
                    op0=ALU.mult, op1=ALU.add)

            # ---- mlp1 (psA) overlapped with mlp2 nt 0-1 chains (ps2a) ----
            pso_cm.__exit__(None, None, None)
            ps2a_cm = tc.tile_pool(name="ps2a", bufs=4, space="PSUM")
            ps2a = ps2a_cm.__enter__()
            pma = {}
            for k in range(2):
                for ci in range(NCH):
                    pma[(k, ci)] = ps2a.tile([128, 512], f32, tag="ps2a",
                                             name=f"pma{k}_{ci}")

            def mlp2_pair(pm, nts, i):
                for k, nt in enumerate(nts):
                    for ci, (off, sz) in enumerate(CH):
                        nc.tensor.matmul(pm[(k, ci)][:, 0:sz],
                                         w2[:, 2 * i:2 * i + 2,
                                            128 * nt:128 * (nt + 1)],
                                         gT[:, 2 * i:2 * i + 2, off:off + sz],
                                         start=(i == 0), stop=(i == 7),
                                         perf_mode=DR)

            def mlp2_epi(pm, nts):
                for k, nt in enumerate(nts):
                    for ci, (off, sz) in enumerate(CH):
                        nc.vector.scalar_tensor_tensor(
                            yT[:, nt, off:off + sz], pm[(k, ci)][:, 0:sz],
                            1.0 / WS, x1b[:, nt, off:off + sz],
                            op0=ALU.mult, op1=ALU.add)
                    for sp in range(2):
                        nc.sync.dma_start(
                            dr4(out_d, 4)[64 * sp:64 * (sp + 1), nt, :],
                            yT[64 * sp:64 * (sp + 1), nt, :])

            for nt in range(16):
                p = ps.tile([128, 2, 512], f32, tag="b2", name=f"m1{nt}")
                for i in range(2):
                    for ci, (off, sz) in enumerate(CH):
                        nc.tensor.matmul(p[:, ci, 0:sz],
                                         w1[:, 2 * i:2 * i + 2,
                                            128 * nt:128 * (nt + 1)],
                                         hT[:, 2 * i:2 * i + 2, off:off + sz],
                                         start=(i == 0), stop=(i == 1),
                                         perf_mode=DR)
                nc.scalar.activation(gT[:, nt, :], pview(p), AF.Gelu,
                                     bias=b1[:, nt:nt + 1], scale=1.0 / WS)
                if has_b2 and nt == 0:
                    for kt in range(4):
                        nc.vector.tensor_scalar(x1b[:, kt, :], x1T[:, kt, :],
                                                b2c[:, kt:kt + 1], None,
                                                op0=ALU.add)
                # mlp2 chains for nt 0-1 trail one gT pair behind mlp1
                if nt >= 3 and nt % 2 == 1:
                    mlp2_pair(pma, (0, 1), (nt - 3) // 2)
            mlp2_pair(pma, (0, 1), 7)
            mlp2_epi(pma, (0, 1))

            ps2a_cm.__exit__(None, None, None)
            psA_cm.__exit__(None, None, None)
            # ---- mlp2 nt 2-3: own psum scope ----
            with tc.tile_pool(name="ps2b", bufs=4, space="PSUM") as ps2b:
                pmb = {}
                for k in range(2):
                    for ci in range(NCH):
                        pmb[(k, ci)] = ps2b.tile([128, 512], f32, tag="ps2b",
                                                 name=f"pmb{k}_{ci}")
                for i in range(8):
                    mlp2_pair(pmb, (2, 3), i)
                mlp2_epi(pmb, (2, 3))

    _split_excess_waits(nc)
    return nc


_prog_cache = {}


def _get_prog(key):
    if key not in _prog_cache:
        _prog_cache[key] = _build(*key)
    return _prog_cache[key]


def _route(xf, gate_w, gate_b):
    """Replicate reference routing: top-2 of xf @ gate_w.T + gate_b."""
    logits = xf @ gate_w.T + gate_b            # [T, NE] fp32
    n = len(logits)
    idx0 = np.argmax(logits, axis=1)
    v0 = logits[np.arange(n), idx0]
    masked = logits.copy()
    masked[np.arange(n), idx0] = -np.inf
    idx1 = np.argmax(masked, axis=1)
    v1 = masked[np.arange(n), idx1]
    m = np.maximum(v0, v1)
    e0 = np.exp(v0 - m)
    e1 = np.exp(v1 - m)
    p0 = e0 / (e0 + e1)
    p1 = e1 / (e0 + e1)
    return np.stack([idx0, idx1], 1), np.stack([p0, p1], 1).astype(np.float32)


def _pack128(a):
    """[R, N] -> [128, (R//128)*N] partition-major layout."""
    R, N = a.shape
    t = R // 128
    return np.ascontiguousarray(
        a.reshape(t, 128, N).transpose(1, 0, 2).reshape(128, t * N))


def kernel(x, gate_w, gate_b, ln1_w, ln1_b, in_proj_w, in_proj_b, out_proj_w,
           out_proj_b, ln2_w, ln2_b, mlp_w1, mlp_b1, mlp_w2, mlp_b2):
    x = np.asarray(x, np.float32)
    B, N, _ = x.shape
    T = B * N
    xf = np.ascontiguousarray(x.reshape(T, E))

    topk_idx, probs = _route(xf, np.asarray(gate_w, np.float32),
                             np.asarray(gate_b, np.float32))

    groups = []          # (token_indices, prob_slice) per core, kk-major
    for kk in range(TOPK):
        for e in range(NE):
            sel = np.nonzero(topk_idx[:, kk] == e)[0]
            groups.append((sel, probs[sel, kk]))
    Cmax = max((len(s) for s, _ in groups), default=128)
    C = max(128, -(-Cmax // 64) * 64)
    KT = -(-C // 128)

    ew = []
    has_vbias = False
    has_b2 = False
    for e in range(NE):
        Wq = np.asarray(in_proj_w[e][0:E], np.float32)
        Wk = np.asarray(in_proj_w[e][E:2 * E], np.float32)
        Wv = np.asarray(in_proj_w[e][2 * E:3 * E], np.float32)
        bq = np.asarray(in_proj_b[e][0:E], np.float32)
        bk = np.asarray(in_proj_b[e][E:2 * E], np.float32)
        bv = np.asarray(in_proj_b[e][2 * E:3 * E], np.float32)
        l1b = np.asarray(ln1_b[e], np.float32)
        l2b = np.asarray(ln2_b[e], np.float32)
        scale = np.float32(1.0) / np.sqrt(np.float32(D))
        wqk = np.concatenate([Wq.T, Wk.T], axis=1) * WS          # [E, 2E]
        bqk = np.concatenate([(Wq @ l1b + bq) * scale, Wk @ l1b + bk])
        vb = (Wv @ l1b + bv) * WS
        w1 = np.asarray(mlp_w1[e], np.float32)
        b2v = np.asarray(mlp_b2[e], np.float32)
        if np.any(vb != 0):
            has_vbias = True
        if np.any(b2v != 0):
            has_b2 = True
        ew.append(dict(
            wqk=_pack128(wqk.astype(np8)),
            bqk=np.ascontiguousarray(bqk, np.float32),
            wv=_pack128((Wv.T * WS).astype(np8)),
            wvb=np.ascontiguousarray(vb.astype(np8)).reshape(1, E),
            wo=_pack128((np.asarray(out_proj_w[e], np.float32).T * WS)
                        .astype(np8)),
            w1=_pack128((w1.T * WS).astype(np8)),
            b1=np.ascontiguousarray(w1 @ l2b + np.asarray(mlp_b1[e],
                                                          np.float32)),
            w2=_pack128((np.asarray(mlp_w2[e], np.float32).T * WS)
                        .astype(np8)),
            b2=b2v,
            bo=np.asarray(out_proj_b[e], np.float32),
            l1w=np.ascontiguousarray(ln1_w[e], np.float32),
            l2w=np.ascontiguousarray(ln2_w[e], np.float32),
        ))

    def colpack(vec, ncol):
        a = np.zeros((128, ncol), np.float32)
        a[:, :] = np.asarray(vec, np.float32).reshape(ncol, 128).T
        return a

    sab = np.zeros((128, 2, 128), np.float32)
    sab[0, 0, 0:64] = 1.0
    sab[32, 0, 64:128] = 1.0
    sab[64, 1, 0:64] = 1.0
    sab[96, 1, 64:128] = 1.0
    selp_np = np.zeros((128, 385), np.float32)
    selp_np[:, 0:256] = sab.reshape(128, 256)
    selp_np[:, 256:384] = 1.0
    selp_np[:, 384] = 1.0 / E

    in_maps = []
    for ci, (sel, _p) in enumerate(groups):
        e = ci % NE
        w = ew[e]
        S = len(sel)
        xg = xf[sel]
        mu_h = xg.mean(1, keepdims=True)
        var_h = ((xg - mu_h) ** 2).mean(1, keepdims=True)
        hg = ((xg - mu_h) / np.sqrt(var_h + EPS) * w["l1w"][None, :])
        hT_np = np.zeros((E, C), np.float32)
        hT_np[:, :S] = hg.T
        xTb_np = np.zeros((E, C), np.float32)
        xTb_np[:, :S] = (xg + w["bo"][None, :]).T
        kbv = np.full((KT * 128,), KEY_PAD_BIAS, np.float32)
        kbv[:max(S, 1)] = 0.0
        consts = np.concatenate([
            colpack(kbv, KT), colpack(w["bqk"], 8), colpack(w["b1"], 16),
            colpack(w["l2w"], 4), colpack(w["b2"], 4)], axis=1)
        im = {"consts": consts, "selp": selp_np,
              "hT": _pack128(hT_np.astype(np8)),
              "xTb": _pack128(xTb_np),
              "wqk": w["wqk"], "wv": w["wv"], "wo": w["wo"],
              "w1": w["w1"], "w2": w["w2"]}
        if has_vbias:
            im["wvb"] = w["wvb"]
        in_maps.append(im)

    nc = _get_prog((C, has_vbias, has_b2))
    res = bass_utils.run_bass_kernel_spmd(
        nc, in_maps, core_ids=list(range(8)),
        trace=bool(int(os.environ.get("KERNEL_TRACE", "0"))))
    kernel.last_exec_time_ns = res.exec_time_ns
    kernel.last_results = res

    out = np.zeros((T, E), np.float32)
    for ci, (sel, p) in enumerate(groups):
        S = len(sel)
        if S == 0:
            continue
        yT = np.asarray(res.results[ci]["yT"], np.float32) \
            .reshape(128, 4, C).transpose(1, 0, 2).reshape(E, C)
        out[sel] += yT[:, :S].T * p[:, None]
    return out.reshape(B, N, E)


# revision 47
# speedup vs baseline: 1.0238x; 1.0238x over previous
"""MoE routing transformer block on 8 trn2 NeuronCores.

Strategy: the reference's (top-k slot kk, expert e) pairs partition the
T=2048 tokens into 8 independent groups (2 slots x 4 experts), each running a
full pre-LN attention+MLP block with attention restricted to the group.
One NeuronCore per (kk, e) pair.

Host: computes the (tiny) router gate + top-2 routing in numpy, gathers each
group's tokens, computes LN1, pre-packs weights into [128, ...] partition
layouts (fp8e4, x16-scaled), launches one SPMD bass kernel on the 8 cores,
then scatter-adds the gate-prob-weighted outputs back.

Device (per core, [feature, token] layout, C = padded group capacity):
  qkT = (WqkT.T @ hT) * (1/128 | 1/16) + bqk   fp8 DoubleRow, bf16 out
  v   = hT.T @ WvT                             fp8 DoubleRow, fp8 out (16v),
                                               col 64 of 66/head = 16.0 (den)
  sT[k,q] = kT_h.T @ qT_h                      bf16, per (head, key-tile)
  expT = exp(sT + key_bias) -> fp8             key_bias kills padded keys
  po[d,q], den[q] = v_aug.T @ expT             fp8 DoubleRow over key pairs
  rden = exp(-ln(den))                         2 ACT ops for all 8 heads
  onorm = po * bcast(rden)                     fp8 out
  x1T = (WoT.T @ onorm)/16 + (xT + bo)         fp8 DoubleRow + one DVE op
  h2T = LN2(x1T) -> fp8                        stats via ones-matmul
  gT = gelu((W1T.T @ h2T)/16 + b1) -> fp8      fp8 DoubleRow, ACT scale fold
  yT = (W2T.T @ gT)/16 + b2 + x1T              fp8 DoubleRow + one DVE op
"""

import os
import numpy as np
import ml_dtypes

import concourse.bass as bass
import concourse.mybir as mybir
import concourse.tile as tile
import concourse.tile_utils as tile_utils
from concourse import bass_utils


def _install_ntff_shim():
    """This image's antenv lacks axon_hooks; synthesize it so trace=True works."""
    import sys as _sys
    import types as _types
    try:
        import antenv.axon_hooks  # noqa: F401
        return
    except ImportError:
        pass
    try:
        from trn_agent_boot.trn_boot import _ntff_profile_via_ctypes
        hook = _ntff_profile_via_ctypes('/opt/axon/libaxon_pjrt.so')
    except Exception:
        hook = None
    mod = _types.ModuleType('antenv.axon_hooks')
    state = {'hook': hook}
    mod.set_axon_ntff_profile_hook = lambda h: state.__setitem__('hook', h)
    mod.get_axon_ntff_profile_hook = lambda: state['hook']
    _sys.modules['antenv.axon_hooks'] = mod
    try:
        import antenv
        antenv.axon_hooks = mod
    except ImportError:
        pass


_install_ntff_shim()

# stale constant leaves 16KiB/partition unused on trn2 (224 phys / 208 usable)
tile_utils.max_sbuf_usage = 208 * 1024

E = 512
H = 8
D = 64
HID = 2048
NE = 4
TOPK = 2
EPS = 1e-5

f32 = mybir.dt.float32
f32r = mybir.dt.float32r
bf16 = mybir.dt.bfloat16
f8 = mybir.dt.float8e4
AF = mybir.ActivationFunctionType
ALU = mybir.AluOpType
DR = mybir.MatmulPerfMode.DoubleRow
np8 = ml_dtypes.float8_e4m3

KEY_PAD_BIAS = -60.0
WS = 16.0          # fp8 weight pre-scale


# ---------------------------------------------------------------------------
# walrus in this container encodes at most one sync wait per instruction;
# Tile's kernel-tail drain can carry several. Split extras onto NoOps.
def _split_excess_waits(nc):
    for fn in nc.m.functions:
        for blk in fn.blocks:
            new_insts = []
            for ins in blk.instructions:
                si = ins.sync_info
                if si is not None and len(si.on_wait) > 1:
                    waits = list(si.on_wait)
                    excess, keep = waits[:-1], waits[-1:]
                    for w in excess:
                        new_insts.append(mybir.InstNoOp(
                            name=f"I-waitsplit-{nc.next_id()}",
                            engine=ins.engine, ins=[], outs=[],
                            sync_info=mybir.SyncInfo(on_wait=[w], on_update=[]),
                        ))
                    si.on_wait = keep
                new_insts.append(ins)
            blk.instructions[:] = new_insts


def _build(C, has_vbias=False, has_b2=False):
    """Build the bass program for group capacity C (multiple of 64)."""
    assert C % 64 == 0
    KT = -(-C // 128)                       # key/token tiles (last may be 64)
    kts = [(i * 128, min(128, C - i * 128)) for i in range(KT)]
    nfull = sum(1 for _, s in kts if s == 128)
    npair = nfull // 2                      # DoubleRow AV pairs
    tails = list(range(2 * npair, KT))      # plain-fp8 AV tiles
    if C <= 512:
        NCH, CSZ = 1, C
    else:
        NCH, CSZ = 2, C // 2
        assert CSZ <= 512
    CH = [(i * CSZ, CSZ) for i in range(NCH)]

    nc = bass.Bass(num_swdge_queues=4)

    NCONST = KT + 8 + 16 + 4 + 4
    consts_d = nc.dram_tensor("consts", [128, NCONST], f32, kind="ExternalInput")
    # f32r constants must come via DMA: memset on f32r fails walrus' ISA check
    selp_d = nc.dram_tensor("selp", [128, 385], f32r, kind="ExternalInput")
    hT_d = nc.dram_tensor("hT", [128, 4 * C], f8, kind="ExternalInput")
    wqk_d = nc.dram_tensor("wqk", [128, 4 * 1024], f8, kind="ExternalInput")
    wv_d = nc.dram_tensor("wv", [128, 4 * 512], f8, kind="ExternalInput")
    wo_d = nc.dram_tensor("wo", [128, 4 * 512], f8, kind="ExternalInput")
    w1_d = nc.dram_tensor("w1", [128, 4 * 2048], f8, kind="ExternalInput")
    w2_d = nc.dram_tensor("w2", [128, 16 * 512], f8, kind="ExternalInput")
    xTb_d = nc.dram_tensor("xTb", [128, 4 * C], f32, kind="ExternalInput")
    if has_vbias:
        wvb_d = nc.dram_tensor("wvb", [1, 512], f8, kind="ExternalInput")
    out_d = nc.dram_tensor("yT", [128, 4 * C], bf16, kind="ExternalOutput")

    def dr4(d, t):
        return d[:].rearrange("p (t c) -> p t c", t=t)

    with tile.TileContext(nc) as tc, nc.allow_low_precision(
            reason="fp8/bf16 rounding on matmul-feeding tiles is intended"):
        with (
            tc.tile_pool(name="const", bufs=1) as cpool,
            tc.tile_pool(name="main", bufs=1) as mpool,
            tc.tile_pool(name="expp", bufs=3) as expp,
            tc.tile_pool(name="scr", bufs=2) as scr,
        ):
            # ---- const tiles (DMAs deferred past the hot path) ----
            cst = cpool.tile([128, NCONST], f32)
            o = [0]
            def _csl(n):
                a = o[0]; o[0] += n
                return cst[:, a:a + n]
            kb, bqk, b1, l2w, b2c = _csl(KT), _csl(8), _csl(16), _csl(4), _csl(4)
            selAB = cpool.tile([128, 2, 128], f32r)
            ones_row = cpool.tile([1, 128], f32r)
            ecolr = cpool.tile([128, 1], f32r)
            ecolb = cpool.tile([128, 1], bf16)
            eps_t = cpool.tile([1, 1], f32)
            nc.vector.memset(eps_t[:], EPS)
            if has_vbias:
                onesb8 = cpool.tile([1, 128], f8)
                nc.vector.memset(onesb8[:], 1.0)

            # ---- big tiles; DMAs in need-order, partition-split for ----
            # ---- queue parallelism                                    ----
            hT = mpool.tile([128, 4, C], f8, tag="hT")
            wqk = mpool.tile([128, 4, 1024], f8, tag="wqk")
            wv = mpool.tile([128, 4, 512], f8, tag="wv")
            wo = mpool.tile([128, 4, 512], f8, tag="wo")
            w1 = mpool.tile([128, 4, 2048], f8, tag="w1")
            w2 = mpool.tile([128, 16, 512], f8, tag="w2")
            xTb = mpool.tile([128, 4, C], f32, tag="xTb")
            qkT = mpool.tile([128, 8, C], bf16, tag="qkT")
            v = mpool.tile([128, KT, 8 * 66], f8, tag="v")
            onormU = mpool.tile([128, 4, C], bf16, tag="onU")
            onormN = mpool.tile([128, 4, C], f8, tag="onN")
            x1T = mpool.tile([128, 4, C], f32r, tag="x1T")
            sq = mpool.tile([128, 4, C], bf16, tag="sq")
            gT = mpool.tile([128, 16, C], f8, tag="gT")
            yT = mpool.tile([128, 4, C], bf16, tag="yT")
            # denominators: head h at partition 32*(h%4), free slot h//4.
            # 1/den = exp(-ln(den)) batched on ACT (2 instrs for all heads);
            # partition bases must be 32-aligned on every engine.
            rden = mpool.tile([128, 2, NCH, CSZ], f32, tag="rden")
            lnden = mpool.tile([128, 2, NCH, CSZ], f32, tag="lnden")
            denr = mpool.tile([128, 2, NCH, CSZ], f32r, tag="denr")
            if has_b2:
                x1b = mpool.tile([128, 4, C], f32r, tag="x1b")
            else:
                x1b = x1T

            def dma_split(t, d, tdim, nsplit, spread=False):
                dv = dr4(d, tdim)
                engs = ((nc.sync, nc.scalar, nc.gpsimd)
                        if spread else (nc.sync,))
                P = 128 // nsplit
                for i in range(nsplit):
                    engs[i % len(engs)].dma_start(t[P * i:P * (i + 1)],
                                                  dv[P * i:P * (i + 1)])

            dma_split(hT, hT_d, 4, 4, spread=True)
            dma_split(wv, wv_d, 4, 2, spread=True)
            dma_split(wqk, wqk_d, 4, 4, spread=True)
            nc.sync.dma_start(cst[:], consts_d[:])
            if has_vbias:
                wvb = cpool.tile([1, 512], f8)
                nc.sync.dma_start(wvb[:], wvb_d[:])
            dma_split(wo, wo_d, 4, 2)
            dma_split(w1, w1_d, 4, 4)
            nc.sync.dma_start(selAB[:], selp_d[:, 0:256].rearrange(
                "p (t c) -> p t c", t=2))
            nc.sync.dma_start(ones_row[:], selp_d[0:1, 256:384])
            nc.sync.dma_start(ecolr[:], selp_d[:, 384:385])
            nc.vector.tensor_copy(ecolb[:], ecolr[:])
            dma_split(w2, w2_d, 16, 4)
            dma_split(xTb, xTb_d, 4, 4)

            # flat init: col 64 of each 66-wide head group is the
            # denominator marker (16.0); data cols 0-63 overwritten by the
            # v-proj copies; col 65 is alignment padding (fp8 slices need
            # even byte offsets)
            # unwritten rden rows must be finite (0 * NaN poisons the
            # broadcast matmul); ln(1)=0 -> exp(0)=1
            nc.gpsimd.memset(rden[:], 1.0)

            nc.gpsimd.memset(v[:], WS)

            def pview(p):
                if NCH == 1:
                    return p[:, 0, 0:C]
                return p[:, :, 0:CSZ]

            psA_cm = tc.tile_pool(name="psA", bufs=2, space="PSUM")
            ps = psA_cm.__enter__()
            pso_cm = tc.tile_pool(name="pso", bufs=2, space="PSUM")
            ps_o = pso_cm.__enter__()

            # ---- v (normal layout, heads in 65-col groups, 16x scaled) ----
            for tt in range(KT):
                toff, tsz = kts[tt]
                p = ps.tile([128, 2, 512], f32, tag="b2", name=f"v{tt}")
                for i in range(2):
                    nc.tensor.matmul(p[0:tsz, 0, :],
                                     hT[:, 2 * i:2 * i + 2, toff:toff + tsz],
                                     wv[:, 2 * i:2 * i + 2, :],
                                     start=(i == 0),
                                     stop=(i == 1 and not has_vbias),
                                     perf_mode=DR)
                if has_vbias:
                    nc.tensor.matmul(p[0:tsz, 0, :], onesb8[0:1, 0:tsz],
                                     wvb[0:1, :], start=False, stop=True)
                nc.vector.tensor_copy(
                    v[0:tsz, tt, :].rearrange("p (h x) -> p h x", x=66)[:, :, 0:64],
                    p[0:tsz, 0, :].rearrange("p (h x) -> p h x", x=64))

            # ---- qkT: k slots first so attention can start early ----
            for nt in (4, 0, 5, 1, 6, 2, 7, 3):
                p = ps.tile([128, 2, 512], f32, tag="b2", name=f"qk{nt}")
                for i in range(2):
                    for ci, (off, sz) in enumerate(CH):
                        nc.tensor.matmul(p[:, ci, 0:sz],
                                         wqk[:, 2 * i:2 * i + 2,
                                             128 * nt:128 * (nt + 1)],
                                         hT[:, 2 * i:2 * i + 2, off:off + sz],
                                         start=(i == 0), stop=(i == 1),
                                         perf_mode=DR)
                if nt < 4:      # q on ACT, k on DVE: conversions in parallel
                    nc.scalar.activation(qkT[:, nt, :], pview(p), AF.Identity,
                                         bias=bqk[:, nt:nt + 1],
                                         scale=1.0 / (WS * 8.0))
                else:
                    nc.vector.tensor_scalar(qkT[:, nt, :], pview(p), 1.0 / WS,
                                            bqk[:, nt:nt + 1],
                                            op0=ALU.mult, op1=ALU.add)

            # ---- attention ----
            for h in range(H):
                bp = 64 * (h % 2)
                j = h // 2
                qT_h = qkT[bp:bp + 64, j, :]
                kT_h = qkT[bp:bp + 64, 4 + j, :]
                et = expp.tile([128, KT, NCH, CSZ], f8, tag="et", name=f"et{h}")
                po = ps_o.tile([66, 2, 512], f32, tag="po", name=f"po{h}")
                def av_pair(i):
                    for ci in range(NCH):
                        nc.tensor.matmul(po[0:66, ci, 0:CSZ],
                                         v[:, 2 * i:2 * i + 2,
                                           66 * h:66 * h + 66],
                                         et[:, 2 * i:2 * i + 2, ci, :],
                                         start=(i == 0),
                                         stop=(i == npair - 1 and not tails),
                                         perf_mode=DR)
                for kt in range(KT):
                    koff, ksz = kts[kt]
                    pss = ps.tile([128, 2, 512], f32, tag="b2",
                                  name=f"s{h}_{kt}")
                    for ci, (off, sz) in enumerate(CH):
                        nc.tensor.matmul(pss[0:ksz, ci, 0:sz],
                                         kT_h[:, koff:koff + ksz],
                                         qT_h[:, off:off + sz],
                                         start=True, stop=True)
                    # AV pair i issued one QK slot after exp(2i+1): the
                    # in-order tensor queue never stalls on ACT
                    if kt >= 2 and kt % 2 == 0 and (kt - 2) // 2 < npair:
                        av_pair((kt - 2) // 2)
                    nc.scalar.activation(et[0:ksz, kt, :, :], pview(pss)[0:ksz],
                                         AF.Exp, bias=kb[0:ksz, kt:kt + 1])
                for i in range(max(0, (KT - 1) // 2), npair):
                    av_pair(i)
                for tx, kt in enumerate(tails):
                    koff, ksz = kts[kt]
                    for ci in range(NCH):
                        nc.tensor.matmul(po[0:66, ci, 0:CSZ],
                                         v[0:ksz, kt, 66 * h:66 * h + 66],
                                         et[0:ksz, kt, ci, :],
                                         start=(npair == 0 and tx == 0),
                                         stop=(tx == len(tails) - 1))
                # unnormalized o + 1/denominator (one fast custom-DVE op)
                nc.vector.tensor_copy(onormU[bp:bp + 64, j, :],
                                      pview(po)[0:64])
                dp = 32 * (h % 4)
                nc.vector.tensor_copy(
                    rden[dp:dp + 1, h // 4, :, :],
                    po[64:65, :, 0:CSZ] if NCH == 2 else po[64:65, 0:1, 0:C])

            # ---- normalize: bcast 1/den over 64 partitions per head ----
            nc.scalar.activation(lnden[:], rden[:], AF.Ln)
            nc.scalar.activation(denr[:], lnden[:], AF.Exp, scale=-1.0)
            for t in range(4):
                rp = ps.tile([128, 2, 512], f32, tag="b2", name=f"rp{t}")
                for ci in range(NCH):
                    nc.tensor.matmul(rp[:, ci, 0:CSZ], selAB[:, t % 2, :],
                                     denr[:, t // 2, ci, :], start=True,
                                     stop=True)
                nc.vector.tensor_mul(onormN[:, t, :], onormU[:, t, :],
                                     pview(rp))

            # ---- out proj + residual, LN2 stats fused in ----
            stm = ps_o.tile([1, 2, 512], f32, tag="po", name="stm")
            stq = ps_o.tile([1, 2, 512], f32, tag="po", name="stq")

            def ln2_stats(kt):
                nc.gpsimd.tensor_mul(sq[:, kt, :], x1T[:, kt, :], x1T[:, kt, :])
                for ci, (off, sz) in enumerate(CH):
                    nc.tensor.matmul(stm[0:1, ci, 0:sz], ecolr[:],
                                     x1T[:, kt, off:off + sz],
                                     start=(kt == 0), stop=(kt == 3))
                    nc.tensor.matmul(stq[0:1, ci, 0:sz], ecolb[:],
                                     sq[:, kt, off:off + sz],
                                     start=(kt == 0), stop=(kt == 3))

            for nt in range(4):
                p = ps.tile([128, 2, 512], f32, tag="b2", name=f"op{nt}")
                for i in range(2):
                    for ci, (off, sz) in enumerate(CH):
                        nc.tensor.matmul(p[:, ci, 0:sz],
                                         wo[:, 2 * i:2 * i + 2,
                                            128 * nt:128 * (nt + 1)],
                                         onormN[:, 2 * i:2 * i + 2, off:off + sz],
                                         start=(i == 0), stop=(i == 1),
                                         perf_mode=DR)
                nc.vector.scalar_tensor_tensor(
                    x1T[:, nt, :], pview(p), 1.0 / WS,
                    xTb[:, nt, :], op0=ALU.mult, op1=ALU.add)
                if nt >= 1:
                    ln2_stats(nt - 1)
            ln2_stats(3)
            def pv1(p):
                return p[0:1, :, 0:CSZ] if NCH == 2 else p[0:1, 0:1, 0:C]
            mu2 = scr.tile([1, C], f32, tag="lnt", name="mu2")
            nc.scalar.activation(mu2[0:1, :], pv1(stm), AF.Square)
            var = scr.tile([1, C], f32, tag="lnt", name="var")
            nc.vector.scalar_tensor_tensor(var[0:1, :], mu2[0:1, :], -1.0,
                                           pv1(stq), op0=ALU.mult, op1=ALU.add)
            lnv = scr.tile([1, C], f32, tag="lnt", name="lnv")
            nc.scalar.activation(lnv[:], var[:], AF.Ln, bias=eps_t[0:1, 0:1])
            rstd = scr.tile([1, C], f32r, tag="lnt", name="rstd")
            nc.scalar.activation(rstd[:], lnv[:], AF.Exp, scale=-0.5)
            mbneg = scr.tile([1, C], f32r, tag="lnt", name="mbneg")
            nc.vector.scalar_tensor_tensor(mbneg[:], pv1(stm), -1.0, rstd[:],
                                           op0=ALU.mult, op1=ALU.mult)
            pA = ps.tile([128, 2, 512], f32, tag="b2", name="lnA")
            pB = ps.tile([128, 2, 512], f32, tag="b2", name="lnB")
            for ci, (off, sz) in enumerate(CH):
                nc.tensor.matmul(pA[:, ci, 0:sz], ones_row[0:1, 0:128],
                                 rstd[0:1, off:off + sz], start=True, stop=True)
                nc.tensor.matmul(pB[:, ci, 0:sz], ones_row[0:1, 0:128],
                                 mbneg[0:1, off:off + sz], start=True, stop=True)
            for kt in range(4):
                tmp = scr.tile([128, C], f32, tag="lntmp", name=f"lt{kt}")
                nc.vector.scalar_tensor_tensor(
                    tmp[:], x1T[:, kt, :], l2w[:, kt:kt + 1], pview(pA),
                    op0=ALU.mult, op1=ALU.mult)
                nc.vector.scalar_tensor_tensor(
                    hT[:, kt, :], pview(pB), l2w[:, kt:kt + 1], tmp[:],
                    op0=ALU.mult, op1=ALU.add)

            # ---- mlp1 (psA) overlapped with mlp2 nt 0-1 chains (ps2a) ----
            pso_cm.__exit__(None, None, None)
            ps2a_cm = tc.tile_pool(name="ps2a", bufs=4, space="PSUM")
            ps2a = ps2a_cm.__enter__()
            pma = {}
            for k in range(2):
                for ci in range(NCH):
                    pma[(k, ci)] = ps2a.tile([128, 512], f32, tag="ps2a",
                                             name=f"pma{k}_{ci}")

            def mlp2_pair(pm, nts, i):
                for k, nt in enumerate(nts):
                    for ci, (off, sz) in enumerate(CH):
                        nc.tensor.matmul(pm[(k, ci)][:, 0:sz],
                                         w2[:, 2 * i:2 * i + 2,
                                            128 * nt:128 * (nt + 1)],
                                         gT[:, 2 * i:2 * i + 2, off:off + sz],
                                         start=(i == 0), stop=(i == 7),
                                         perf_mode=DR)

            def mlp2_epi(pm, nts):
                for k, nt in enumerate(nts):
                    for ci, (off, sz) in enumerate(CH):
                        nc.vector.scalar_tensor_tensor(
                            yT[:, nt, off:off + sz], pm[(k, ci)][:, 0:sz],
                            1.0 / WS, x1b[:, nt, off:off + sz],
                            op0=ALU.mult, op1=ALU.add)
                    for sp in range(2):
                        nc.sync.dma_start(
                            dr4(out_d, 4)[64 * sp:64 * (sp + 1), nt, :],
                            yT[64 * sp:64 * (sp + 1), nt, :])

            for nt in range(16):
                p = ps.tile([128, 2, 512], f32, tag="b2", name=f"m1{nt}")
                for i in range(2):
                    for ci, (off, sz) in enumerate(CH):
                        nc.tensor.matmul(p[:, ci, 0:sz],
                                         w1[:, 2 * i:2 * i + 2,
                                            128 * nt:128 * (nt + 1)],
                                         hT[:, 2 * i:2 * i + 2, off:off + sz],
                                         start=(i == 0), stop=(i == 1),
                                         perf_mode=DR)
                nc.scalar.activation(gT[:, nt, :], pview(p), AF.Gelu,
                                     bias=b1[:, nt:nt + 1], scale=1.0 / WS)
                if has_b2 and nt == 0:
                    for kt in range(4):
                        nc.vector.tensor_scalar(x1b[:, kt, :], x1T[:, kt, :],
                                                b2c[:, kt:kt + 1], None,
                                                op0=ALU.add)
                # mlp2 chains for nt 0-1 trail one gT pair behind mlp1
                if nt >= 3 and nt % 2 == 1:
                    mlp2_pair(pma, (0, 1), (nt - 3) // 2)
            mlp2_pair(pma, (0, 1), 7)
            mlp2_epi(pma, (0, 1))

            ps2a_cm.__exit__(None, None, None)
            psA_cm.__exit__(None, None, None)
            # ---- mlp2 nt 2-3: own psum scope ----
            with tc.tile_pool(name="ps2b", bufs=4, space="PSUM") as ps2b:
                pmb = {}
                for k in range(2):
                    for ci in range(NCH):
                        pmb[(k, ci)] = ps2b.tile([128, 512], f32, tag="ps2b",
                                                 name=f"pmb{k}_{ci}")
                for i in range(8):
                    mlp2_pair(pmb, (2, 3), i)
                mlp2_epi(pmb, (2, 3))

    _split_excess_waits(nc)
    return nc


_prog_cache = {}


def _get_prog(key):
    if key not in _prog_cache:
        _prog_cache[key] = _build(*key)
    return _prog_cache[key]


def _route(xf, gate_w, gate_b):
    """Replicate reference routing: top-2 of xf @ gate_w.T + gate_b."""
    logits = xf @ gate_w.T + gate_b            # [T, NE] fp32
    n = len(logits)
    idx0 = np.argmax(logits, axis=1)
    v0 = logits[np.arange(n), idx0]
    masked = logits.copy()
    masked[np.arange(n), idx0] = -np.inf
    idx1 = np.argmax(masked, axis=1)
    v1 = masked[np.arange(n), idx1]
    m = np.maximum(v0, v1)
    e0 = np.exp(v0 - m)
    e1 = np.exp(v1 - m)
    p0 = e0 / (e0 + e1)
    p1 = e1 / (e0 + e1)
    return np.stack([idx0, idx1], 1), np.stack([p0, p1], 1).astype(np.float32)


def _pack128(a):
    """[R, N] -> [128, (R//128)*N] partition-major layout."""
    R, N = a.shape
    t = R // 128
    return np.ascontiguousarray(
        a.reshape(t, 128, N).transpose(1, 0, 2).reshape(128, t * N))


def kernel(x, gate_w, gate_b, ln1_w, ln1_b, in_proj_w, in_proj_b, out_proj_w,
           out_proj_b, ln2_w, ln2_b, mlp_w1, mlp_b1, mlp_w2, mlp_b2):
    x = np.asarray(x, np.float32)
    B, N, _ = x.shape
    T = B * N
    xf = np.ascontiguousarray(x.reshape(T, E))

    topk_idx, probs = _route(xf, np.asarray(gate_w, np.float32),
                             np.asarray(gate_b, np.float32))

    groups = []          # (token_indices, prob_slice) per core, kk-major
    for kk in range(TOPK):
        for e in range(NE):
            sel = np.nonzero(topk_idx[:, kk] == e)[0]
            groups.append((sel, probs[sel, kk]))
    Cmax = max((len(s) for s, _ in groups), default=128)
    C = max(128, -(-Cmax // 64) * 64)
    KT = -(-C // 128)

    ew = []
    has_vbias = False
    has_b2 = False
    for e in range(NE):
        Wq = np.asarray(in_proj_w[e][0:E], np.float32)
        Wk = np.asarray(in_proj_w[e][E:2 * E], np.float32)
        Wv = np.asarray(in_proj_w[e][2 * E:3 * E], np.float32)
        bq = np.asarray(in_proj_b[e][0:E], np.float32)
        bk = np.asarray(in_proj_b[e][E:2 * E], np.float32)
        bv = np.asarray(in_proj_b[e][2 * E:3 * E], np.float32)
        l1b = np.asarray(ln1_b[e], np.float32)
        l2b = np.asarray(ln2_b[e], np.float32)
        scale = np.float32(1.0) / np.sqrt(np.float32(D))
        wqk = np.concatenate([Wq.T, Wk.T], axis=1) * WS          # [E, 2E]
        bqk = np.concatenate([(Wq @ l1b + bq) * scale, Wk @ l1b + bk])
        vb = (Wv @ l1b + bv) * WS
        w1 = np.asarray(mlp_w1[e], np.float32)
        b2v = np.asarray(mlp_b2[e], np.float32)
        if np.any(vb != 0):
            has_vbias = True
        if np.any(b2v != 0):
            has_b2 = True
        ew.append(dict(
            wqk=_pack128(wqk.astype(np8)),
            bqk=np.ascontiguousarray(bqk, np.float32),
            wv=_pack128((Wv.T * WS).astype(np8)),
            wvb=np.ascontiguousarray(vb.astype(np8)).reshape(1, E),
            wo=_pack128((np.asarray(out_proj_w[e], np.float32).T * WS)
                        .astype(np8)),
            w1=_pack128((w1.T * WS).astype(np8)),
            b1=np.ascontiguousarray(w1 @ l2b + np.asarray(mlp_b1[e],
                                                          np.float32)),
            w2=_pack128((np.asarray(mlp_w2[e], np.float32).T * WS)
                        .astype(np8)),
            b2=b2v,
            bo=np.asarray(out_proj_b[e], np.float32),
            l1w=np.ascontiguousarray(ln1_w[e], np.float32),
            l2w=np.ascontiguousarray(ln2_w[e], np.float32),
        ))

    def colpack(vec, ncol):
        a = np.zeros((128, ncol), np.float32)
        a[:, :] = np.asarray(vec, np.float32).reshape(ncol, 128).T
        return a

    sab = np.zeros((128, 2, 128), np.float32)
    sab[0, 0, 0:64] = 1.0
    sab[32, 0, 64:128] = 1.0
    sab[64, 1, 0:64] = 1.0
    sab[96, 1, 64:128] = 1.0
    selp_np = np.zeros((128, 385), np.float32)
    selp_np[:, 0:256] = sab.reshape(128, 256)
    selp_np[:, 256:384] = 1.0
    selp_np[:, 384] = 1.0 / E

    in_maps = []
    for ci, (sel, _p) in enumerate(groups):
        e = ci % NE
        w = ew[e]
        S = len(sel)
        xg = xf[sel]
        mu_h = xg.mean(1, keepdims=True)
        var_h = ((xg - mu_h) ** 2).mean(1, keepdims=True)
        hg = ((xg - mu_h) / np.sqrt(var_h + EPS) * w["l1w"][None, :])
        hT_np = np.zeros((E, C), np.float32)
        hT_np[:, :S] = hg.T
        xTb_np = np.zeros((E, C), np.float32)
        xTb_np[:, :S] = (xg + w["bo"][None, :]).T
        kbv = np.full((KT * 128,), KEY_PAD_BIAS, np.float32)
        kbv[:max(S, 1)] = 0.0
        consts = np.concatenate([
            colpack(kbv, KT), colpack(w["bqk"], 8), colpack(w["b1"], 16),
            colpack(w["l2w"], 4), colpack(w["b2"], 4)], axis=1)
        im = {"consts": consts, "selp": selp_np,
              "hT": _pack128(hT_np.astype(np8)),
              "xTb": _pack128(xTb_np),
              "wqk": w["wqk"], "wv": w["wv"], "wo": w["wo"],
              "w1": w["w1"], "w2": w["w2"]}
        if has_vbias:
            im["wvb"] = w["wvb"]
        in_maps.append(im)

    nc = _get_prog((C, has_vbias, has_b2))
    res = bass_utils.run_bass_kernel_spmd(
        nc, in_maps, core_ids=list(range(8)),
        trace=bool(int(os.environ.get("KERNEL_TRACE", "0"))))
    kernel.last_exec_time_ns = res.exec_time_ns
    kernel.last_results = res

    out = np.zeros((T, E), np.float32)
    for ci, (sel, p) in enumerate(groups):
        S = len(sel)
        if S == 0:
            continue
        yT = np.asarray(res.results[ci]["yT"], np.float32) \
            .reshape(128, 4, C).transpose(1, 0, 2).reshape(E, C)
        out[sel] += yT[:, :S].T * p[:, None]
    return out.reshape(B, N, E)


# revision 49
# speedup vs baseline: 1.0242x; 1.0005x over previous
"""MoE routing transformer block on 8 trn2 NeuronCores.

Strategy: the reference's (top-k slot kk, expert e) pairs partition the
T=2048 tokens into 8 independent groups (2 slots x 4 experts), each running a
full pre-LN attention+MLP block with attention restricted to the group.
One NeuronCore per (kk, e) pair.

Host: computes the (tiny) router gate + top-2 routing in numpy, gathers each
group's tokens, computes LN1, pre-packs weights into [128, ...] partition
layouts (fp8e4, x16-scaled), launches one SPMD bass kernel on the 8 cores,
then scatter-adds the gate-prob-weighted outputs back.

Device (per core, [feature, token] layout, C = padded group capacity):
  qkT = (WqkT.T @ hT) * (1/128 | 1/16) + bqk   fp8 DoubleRow, bf16 out
  v   = hT.T @ WvT                             fp8 DoubleRow, fp8 out (16v),
                                               col 64 of 66/head = 16.0 (den)
  sT[k,q] = kT_h.T @ qT_h                      bf16, per (head, key-tile)
  expT = exp(sT + key_bias) -> fp8             key_bias kills padded keys
  po[d,q], den[q] = v_aug.T @ expT             fp8 DoubleRow over key pairs
  rden = exp(-ln(den))                         2 ACT ops for all 8 heads
  onorm = po * bcast(rden)                     fp8 out
  x1T = (WoT.T @ onorm)/16 + (xT + bo)         fp8 DoubleRow + one DVE op
  h2T = LN2(x1T) -> fp8                        stats via ones-matmul
  gT = gelu((W1T.T @ h2T)/16 + b1) -> fp8      fp8 DoubleRow, ACT scale fold
  yT = (W2T.T @ gT)/16 + b2 + x1T              fp8 DoubleRow + one DVE op
"""

import os
import numpy as np
import ml_dtypes

import concourse.bass as bass
import concourse.mybir as mybir
import concourse.tile as tile
import concourse.tile_utils as tile_utils
from concourse import bass_utils


def _install_ntff_shim():
    """This image's antenv lacks axon_hooks; synthesize it so trace=True works."""
    import sys as _sys
    import types as _types
    try:
        import antenv.axon_hooks  # noqa: F401
        return
    except ImportError:
        pass
    try:
        from trn_agent_boot.trn_boot import _ntff_profile_via_ctypes
        hook = _ntff_profile_via_ctypes('/opt/axon/libaxon_pjrt.so')
    except Exception:
        hook = None
    mod = _types.ModuleType('antenv.axon_hooks')
    state = {'hook': hook}
    mod.set_axon_ntff_profile_hook = lambda h: state.__setitem__('hook', h)
    mod.get_axon_ntff_profile_hook = lambda: state['hook']
    _sys.modules['antenv.axon_hooks'] = mod
    try:
        import antenv
        antenv.axon_hooks = mod
    except ImportError:
        pass


_install_ntff_shim()

# stale constant leaves 16KiB/partition unused on trn2 (224 phys / 208 usable)
tile_utils.max_sbuf_usage = 208 * 1024

E = 512
H = 8
D = 64
HID = 2048
NE = 4
TOPK = 2
EPS = 1e-5

f32 = mybir.dt.float32
f32r = mybir.dt.float32r
bf16 = mybir.dt.bfloat16
f8 = mybir.dt.float8e4
AF = mybir.ActivationFunctionType
ALU = mybir.AluOpType
DR = mybir.MatmulPerfMode.DoubleRow
np8 = ml_dtypes.float8_e4m3

KEY_PAD_BIAS = -60.0
WS = 16.0          # fp8 weight pre-scale


# ---------------------------------------------------------------------------
# walrus in this container encodes at most one sync wait per instruction;
# Tile's kernel-tail drain can carry several. Split extras onto NoOps.
def _split_excess_waits(nc):
    for fn in nc.m.functions:
        for blk in fn.blocks:
            new_insts = []
            for ins in blk.instructions:
                si = ins.sync_info
                if si is not None and len(si.on_wait) > 1:
                    waits = list(si.on_wait)
                    excess, keep = waits[:-1], waits[-1:]
                    for w in excess:
                        new_insts.append(mybir.InstNoOp(
                            name=f"I-waitsplit-{nc.next_id()}",
                            engine=ins.engine, ins=[], outs=[],
                            sync_info=mybir.SyncInfo(on_wait=[w], on_update=[]),
                        ))
                    si.on_wait = keep
                new_insts.append(ins)
            blk.instructions[:] = new_insts


def _build(C, has_vbias=False, has_b2=False):
    """Build the bass program for group capacity C (multiple of 64)."""
    assert C % 64 == 0
    KT = -(-C // 128)                       # key/token tiles (last may be 64)
    kts = [(i * 128, min(128, C - i * 128)) for i in range(KT)]
    nfull = sum(1 for _, s in kts if s == 128)
    npair = nfull // 2                      # DoubleRow AV pairs
    tails = list(range(2 * npair, KT))      # plain-fp8 AV tiles
    if C <= 512:
        NCH, CSZ = 1, C
    else:
        NCH, CSZ = 2, C // 2
        assert CSZ <= 512
    CH = [(i * CSZ, CSZ) for i in range(NCH)]

    nc = bass.Bass(num_swdge_queues=4)

    NCONST = KT + 8 + 16 + 4 + 4
    consts_d = nc.dram_tensor("consts", [128, NCONST], f32, kind="ExternalInput")
    # f32r constants must come via DMA: memset on f32r fails walrus' ISA check
    selp_d = nc.dram_tensor("selp", [128, 385], f32r, kind="ExternalInput")
    hT_d = nc.dram_tensor("hT", [128, 4 * C], f8, kind="ExternalInput")
    wqk_d = nc.dram_tensor("wqk", [128, 4 * 1024], f8, kind="ExternalInput")
    wv_d = nc.dram_tensor("wv", [128, 4 * 512], f8, kind="ExternalInput")
    wo_d = nc.dram_tensor("wo", [128, 4 * 512], f8, kind="ExternalInput")
    w1_d = nc.dram_tensor("w1", [128, 4 * 2048], f8, kind="ExternalInput")
    w2_d = nc.dram_tensor("w2", [128, 16 * 512], f8, kind="ExternalInput")
    xTb_d = nc.dram_tensor("xTb", [128, 4 * C], f32, kind="ExternalInput")
    if has_vbias:
        wvb_d = nc.dram_tensor("wvb", [1, 512], f8, kind="ExternalInput")
    out_d = nc.dram_tensor("yT", [128, 4 * C], bf16, kind="ExternalOutput")

    def dr4(d, t):
        return d[:].rearrange("p (t c) -> p t c", t=t)

    with tile.TileContext(nc) as tc, nc.allow_low_precision(
            reason="fp8/bf16 rounding on matmul-feeding tiles is intended"):
        with (
            tc.tile_pool(name="const", bufs=1) as cpool,
            tc.tile_pool(name="main", bufs=1) as mpool,
            tc.tile_pool(name="expp", bufs=3) as expp,
            tc.tile_pool(name="scr", bufs=2) as scr,
        ):
            # ---- const tiles (DMAs deferred past the hot path) ----
            cst = cpool.tile([128, NCONST], f32)
            o = [0]
            def _csl(n):
                a = o[0]; o[0] += n
                return cst[:, a:a + n]
            kb, bqk, b1, l2w, b2c = _csl(KT), _csl(8), _csl(16), _csl(4), _csl(4)
            selAB = cpool.tile([128, 2, 128], f32r)
            ones_row = cpool.tile([1, 128], f32r)
            ecolr = cpool.tile([128, 1], f32r)
            ecolb = cpool.tile([128, 1], bf16)
            eps_t = cpool.tile([1, 1], f32)
            nc.vector.memset(eps_t[:], EPS)
            if has_vbias:
                onesb8 = cpool.tile([1, 128], f8)
                nc.vector.memset(onesb8[:], 1.0)

            # ---- big tiles; DMAs in need-order, partition-split for ----
            # ---- queue parallelism                                    ----
            hT = mpool.tile([128, 4, C], f8, tag="hT")
            wqk = mpool.tile([128, 4, 1024], f8, tag="wqk")
            wv = mpool.tile([128, 4, 512], f8, tag="wv")
            wo = mpool.tile([128, 4, 512], f8, tag="wo")
            w1 = mpool.tile([128, 4, 2048], f8, tag="w1")
            w2 = mpool.tile([128, 16, 512], f8, tag="w2")
            xTb = mpool.tile([128, 4, C], f32, tag="xTb")
            qkT = mpool.tile([128, 8, C], bf16, tag="qkT")
            v = mpool.tile([128, KT, 8 * 66], f8, tag="v")
            onormU = mpool.tile([128, 4, C], bf16, tag="onU")
            onormN = mpool.tile([128, 4, C], f8, tag="onN")
            x1T = mpool.tile([128, 4, C], f32r, tag="x1T")
            sq = mpool.tile([128, 4, C], bf16, tag="sq")
            gT = mpool.tile([128, 16, C], f8, tag="gT")
            yT = mpool.tile([128, 4, C], bf16, tag="yT")
            # denominators: head h at partition 32*(h%4), free slot h//4.
            # 1/den = exp(-ln(den)) batched on ACT (2 instrs for all heads);
            # partition bases must be 32-aligned on every engine.
            rden = mpool.tile([128, 2, NCH, CSZ], f32, tag="rden")
            lnden = mpool.tile([128, 2, NCH, CSZ], f32, tag="lnden")
            denr = mpool.tile([128, 2, NCH, CSZ], f32r, tag="denr")
            if has_b2:
                x1b = mpool.tile([128, 4, C], f32r, tag="x1b")
            else:
                x1b = x1T

            def dma_split(t, d, tdim, nsplit, spread=False):
                dv = dr4(d, tdim)
                engs = ((nc.sync, nc.scalar, nc.gpsimd)
                        if spread else (nc.sync,))
                P = 128 // nsplit
                for i in range(nsplit):
                    engs[i % len(engs)].dma_start(t[P * i:P * (i + 1)],
                                                  dv[P * i:P * (i + 1)])

            dma_split(hT, hT_d, 4, 4, spread=True)
            dma_split(wv, wv_d, 4, 2, spread=True)
            dma_split(wqk, wqk_d, 4, 4, spread=True)
            nc.sync.dma_start(cst[:], consts_d[:])
            if has_vbias:
                wvb = cpool.tile([1, 512], f8)
                nc.sync.dma_start(wvb[:], wvb_d[:])
            dma_split(wo, wo_d, 4, 2)
            dma_split(w1, w1_d, 4, 4)
            nc.sync.dma_start(selAB[:], selp_d[:, 0:256].rearrange(
                "p (t c) -> p t c", t=2))
            nc.sync.dma_start(ones_row[:], selp_d[0:1, 256:384])
            nc.sync.dma_start(ecolr[:], selp_d[:, 384:385])
            nc.vector.tensor_copy(ecolb[:], ecolr[:])
            dma_split(w2, w2_d, 16, 4)
            dma_split(xTb, xTb_d, 4, 4)

            # flat init: col 64 of each 66-wide head group is the
            # denominator marker (16.0); data cols 0-63 overwritten by the
            # v-proj copies; col 65 is alignment padding (fp8 slices need
            # even byte offsets)
            # unwritten rden rows must be finite (0 * NaN poisons the
            # broadcast matmul); ln(1)=0 -> exp(0)=1
            nc.gpsimd.memset(rden[:], 1.0)

            nc.gpsimd.memset(v[:], WS)

            def pview(p):
                if NCH == 1:
                    return p[:, 0, 0:C]
                return p[:, :, 0:CSZ]

            psA_cm = tc.tile_pool(name="psA", bufs=2, space="PSUM")
            ps = psA_cm.__enter__()
            pso_cm = tc.tile_pool(name="pso", bufs=2, space="PSUM")
            ps_o = pso_cm.__enter__()

            # ---- v (normal layout, heads in 65-col groups, 16x scaled) ----
            for tt in range(KT):
                toff, tsz = kts[tt]
                p = ps.tile([128, 2, 512], f32, tag="b2", name=f"v{tt}")
                for i in range(2):
                    nc.tensor.matmul(p[0:tsz, 0, :],
                                     hT[:, 2 * i:2 * i + 2, toff:toff + tsz],
                                     wv[:, 2 * i:2 * i + 2, :],
                                     start=(i == 0),
                                     stop=(i == 1 and not has_vbias),
                                     perf_mode=DR)
                if has_vbias:
                    nc.tensor.matmul(p[0:tsz, 0, :], onesb8[0:1, 0:tsz],
                                     wvb[0:1, :], start=False, stop=True)
                nc.vector.tensor_copy(
                    v[0:tsz, tt, :].rearrange("p (h x) -> p h x", x=66)[:, :, 0:64],
                    p[0:tsz, 0, :].rearrange("p (h x) -> p h x", x=64))

            # ---- qkT: k slots first so attention can start early ----
            for nt in (4, 0, 5, 1, 6, 2, 7, 3):
                p = ps.tile([128, 2, 512], f32, tag="b2", name=f"qk{nt}")
                for i in range(2):
                    for ci, (off, sz) in enumerate(CH):
                        nc.tensor.matmul(p[:, ci, 0:sz],
                                         wqk[:, 2 * i:2 * i + 2,
                                             128 * nt:128 * (nt + 1)],
                                         hT[:, 2 * i:2 * i + 2, off:off + sz],
                                         start=(i == 0), stop=(i == 1),
                                         perf_mode=DR)
                if nt < 4:      # q on ACT, k on DVE: conversions in parallel
                    nc.scalar.activation(qkT[:, nt, :], pview(p), AF.Identity,
                                         bias=bqk[:, nt:nt + 1],
                                         scale=1.0 / (WS * 8.0))
                else:
                    nc.vector.tensor_scalar(qkT[:, nt, :], pview(p), 1.0 / WS,
                                            bqk[:, nt:nt + 1],
                                            op0=ALU.mult, op1=ALU.add)

            # ---- attention ----
            for h in range(H):
                bp = 64 * (h % 2)
                j = h // 2
                qT_h = qkT[bp:bp + 64, j, :]
                kT_h = qkT[bp:bp + 64, 4 + j, :]
                et = expp.tile([128, KT, NCH, CSZ], f8, tag="et", name=f"et{h}")
                po = ps_o.tile([66, 2, 512], f32, tag="po", name=f"po{h}")
                def av_pair(i):
                    for ci in range(NCH):
                        nc.tensor.matmul(po[0:66, ci, 0:CSZ],
                                         v[:, 2 * i:2 * i + 2,
                                           66 * h:66 * h + 66],
                                         et[:, 2 * i:2 * i + 2, ci, :],
                                         start=(i == 0),
                                         stop=(i == npair - 1 and not tails),
                                         perf_mode=DR)
                for kt in range(KT):
                    koff, ksz = kts[kt]
                    pss = ps.tile([128, 2, 512], f32, tag="b2",
                                  name=f"s{h}_{kt}")
                    for ci, (off, sz) in enumerate(CH):
                        nc.tensor.matmul(pss[0:ksz, ci, 0:sz],
                                         kT_h[:, koff:koff + ksz],
                                         qT_h[:, off:off + sz],
                                         start=True, stop=True)
                    # AV pair i issued one QK slot after exp(2i+1): the
                    # in-order tensor queue never stalls on ACT
                    if kt >= 2 and kt % 2 == 0 and (kt - 2) // 2 < npair:
                        av_pair((kt - 2) // 2)
                    nc.scalar.activation(et[0:ksz, kt, :, :], pview(pss)[0:ksz],
                                         AF.Exp, bias=kb[0:ksz, kt:kt + 1])
                for i in range(max(0, (KT - 1) // 2), npair):
                    av_pair(i)
                for tx, kt in enumerate(tails):
                    koff, ksz = kts[kt]
                    for ci in range(NCH):
                        nc.tensor.matmul(po[0:66, ci, 0:CSZ],
                                         v[0:ksz, kt, 66 * h:66 * h + 66],
                                         et[0:ksz, kt, ci, :],
                                         start=(npair == 0 and tx == 0),
                                         stop=(tx == len(tails) - 1))
                # unnormalized o + 1/denominator (one fast custom-DVE op)
                nc.vector.tensor_copy(onormU[bp:bp + 64, j, :],
                                      pview(po)[0:64])
                dp = 32 * (h % 4)
                nc.vector.tensor_copy(
                    rden[dp:dp + 1, h // 4, :, :],
                    po[64:65, :, 0:CSZ] if NCH == 2 else po[64:65, 0:1, 0:C])

            # ---- normalize: bcast 1/den over 64 partitions per head ----
            nc.scalar.activation(lnden[:], rden[:], AF.Ln)
            nc.scalar.activation(denr[:], lnden[:], AF.Exp, scale=-1.0)
            for t in range(4):
                rp = ps.tile([128, 2, 512], f32, tag="b2", name=f"rp{t}")
                for ci in range(NCH):
                    nc.tensor.matmul(rp[:, ci, 0:CSZ], selAB[:, t % 2, :],
                                     denr[:, t // 2, ci, :], start=True,
                                     stop=True)
                nc.vector.tensor_mul(onormN[:, t, :], onormU[:, t, :],
                                     pview(rp))

            # ---- out proj + residual, LN2 stats fused in ----
            stm = ps_o.tile([1, 2, 512], f32, tag="po", name="stm")
            stq = ps_o.tile([1, 2, 512], f32, tag="po", name="stq")

            def ln2_stats(kt):
                nc.gpsimd.tensor_mul(sq[:, kt, :], x1T[:, kt, :], x1T[:, kt, :])
                for ci, (off, sz) in enumerate(CH):
                    nc.tensor.matmul(stm[0:1, ci, 0:sz], ecolr[:],
                                     x1T[:, kt, off:off + sz],
                                     start=(kt == 0), stop=(kt == 3))
                    nc.tensor.matmul(stq[0:1, ci, 0:sz], ecolb[:],
                                     sq[:, kt, off:off + sz],
                                     start=(kt == 0), stop=(kt == 3))

            for nt in range(4):
                p = ps.tile([128, 2, 512], f32, tag="b2", name=f"op{nt}")
                for i in range(2):
                    for ci, (off, sz) in enumerate(CH):
                        nc.tensor.matmul(p[:, ci, 0:sz],
                                         wo[:, 2 * i:2 * i + 2,
                                            128 * nt:128 * (nt + 1)],
                                         onormN[:, 2 * i:2 * i + 2, off:off + sz],
                                         start=(i == 0), stop=(i == 1),
                                         perf_mode=DR)
                nc.vector.scalar_tensor_tensor(
                    x1T[:, nt, :], pview(p), 1.0 / WS,
                    xTb[:, nt, :], op0=ALU.mult, op1=ALU.add)
                if nt >= 1:
                    ln2_stats(nt - 1)
            ln2_stats(3)
            def pv1(p):
                return p[0:1, :, 0:CSZ] if NCH == 2 else p[0:1, 0:1, 0:C]
            mu2 = scr.tile([1, C], f32, tag="lnt", name="mu2")
            nc.scalar.activation(mu2[0:1, :], pv1(stm), AF.Square)
            var = scr.tile([1, C], f32, tag="lnt", name="var")
            nc.vector.scalar_tensor_tensor(var[0:1, :], mu2[0:1, :], -1.0,
                                           pv1(stq), op0=ALU.mult, op1=ALU.add)
            lnv = scr.tile([1, C], f32, tag="lnt", name="lnv")
            nc.scalar.activation(lnv[:], var[:], AF.Ln, bias=eps_t[0:1, 0:1])
            rstd = scr.tile([1, C], f32r, tag="lnt", name="rstd")
            nc.scalar.activation(rstd[:], lnv[:], AF.Exp, scale=-0.5)
            mbneg = scr.tile([1, C], f32r, tag="lnt", name="mbneg")
            nc.vector.scalar_tensor_tensor(mbneg[:], pv1(stm), -1.0, rstd[:],
                                           op0=ALU.mult, op1=ALU.mult)
            pA = ps.tile([128, 2, 512], f32, tag="b2", name="lnA")
            pB = ps.tile([128, 2, 512], f32, tag="b2", name="lnB")
            for ci, (off, sz) in enumerate(CH):
                nc.tensor.matmul(pA[:, ci, 0:sz], ones_row[0:1, 0:128],
                                 rstd[0:1, off:off + sz], start=True, stop=True)
                nc.tensor.matmul(pB[:, ci, 0:sz], ones_row[0:1, 0:128],
                                 mbneg[0:1, off:off + sz], start=True, stop=True)
            for kt in range(4):
                tmp = scr.tile([128, C], f32, tag="lntmp", name=f"lt{kt}")
                nc.vector.scalar_tensor_tensor(
                    tmp[:], x1T[:, kt, :], l2w[:, kt:kt + 1], pview(pA),
                    op0=ALU.mult, op1=ALU.mult)
                nc.vector.scalar_tensor_tensor(
                    hT[:, kt, :], pview(pB), l2w[:, kt:kt + 1], tmp[:],
                    op0=ALU.mult, op1=ALU.add)

            # ---- mlp1 (psA) overlapped with mlp2 nt 0-1 chains (ps2a) ----
            pso_cm.__exit__(None, None, None)
            ps2a_cm = tc.tile_pool(name="ps2a", bufs=4, space="PSUM")
            ps2a = ps2a_cm.__enter__()
            pma = {}
            for k in range(2):
                for ci in range(NCH):
                    pma[(k, ci)] = ps2a.tile([128, 512], f32, tag="ps2a",
                                             name=f"pma{k}_{ci}")

            def mlp2_pair(pm, nts, i):
                for k, nt in enumerate(nts):
                    for ci, (off, sz) in enumerate(CH):
                        nc.tensor.matmul(pm[(k, ci)][:, 0:sz],
                                         w2[:, 2 * i:2 * i + 2,
                                            128 * nt:128 * (nt + 1)],
                                         gT[:, 2 * i:2 * i + 2, off:off + sz],
                                         start=(i == 0), stop=(i == 7),
                                         perf_mode=DR)

            def mlp2_epi(pm, nts):
                for k, nt in enumerate(nts):
                    for ci, (off, sz) in enumerate(CH):
                        nc.vector.scalar_tensor_tensor(
                            yT[:, nt, off:off + sz], pm[(k, ci)][:, 0:sz],
                            1.0 / WS, x1b[:, nt, off:off + sz],
                            op0=ALU.mult, op1=ALU.add)
                    for sp in range(2):
                        nc.sync.dma_start(
                            dr4(out_d, 4)[64 * sp:64 * (sp + 1), nt, :],
                            yT[64 * sp:64 * (sp + 1), nt, :])

            for nt in range(16):
                p = ps.tile([128, 2, 512], f32, tag="b2", name=f"m1{nt}")
                for i in range(2):
                    for ci, (off, sz) in enumerate(CH):
                        nc.tensor.matmul(p[:, ci, 0:sz],
                                         w1[:, 2 * i:2 * i + 2,
                                            128 * nt:128 * (nt + 1)],
                                         hT[:, 2 * i:2 * i + 2, off:off + sz],
                                         start=(i == 0), stop=(i == 1),
                                         perf_mode=DR)
                nc.scalar.activation(gT[:, nt, :], pview(p), AF.Gelu,
                                     bias=b1[:, nt:nt + 1], scale=1.0 / WS)
                if has_b2 and nt == 0:
                    for kt in range(4):
                        nc.vector.tensor_scalar(x1b[:, kt, :], x1T[:, kt, :],
                                                b2c[:, kt:kt + 1], None,
                                                op0=ALU.add)
                # mlp2 chains for nt 0-1 trail one gT pair behind mlp1
                if nt >= 3 and nt % 2 == 1:
                    mlp2_pair(pma, (0, 1), (nt - 3) // 2)
            mlp2_pair(pma, (0, 1), 7)
            mlp2_epi(pma, (0, 1))

            ps2a_cm.__exit__(None, None, None)
            psA_cm.__exit__(None, None, None)
            # ---- mlp2 nt 2-3: own psum scope ----
            with tc.tile_pool(name="ps2b", bufs=4, space="PSUM") as ps2b:
                pmb = {}
                for k in range(2):
                    for ci in range(NCH):
                        pmb[(k, ci)] = ps2b.tile([128, 512], f32, tag="ps2b",
                                                 name=f"pmb{k}_{ci}")
                for i in range(8):
                    mlp2_pair(pmb, (2, 3), i)
                mlp2_epi(pmb, (2, 3))

    _split_excess_waits(nc)
    return nc


_prog_cache = {}


def _get_prog(key):
    if key not in _prog_cache:
        _prog_cache[key] = _build(*key)
    return _prog_cache[key]


def _route(xf, gate_w, gate_b):
    """Replicate reference routing: top-2 of xf @ gate_w.T + gate_b."""
    logits = xf @ gate_w.T + gate_b            # [T, NE] fp32
    n = len(logits)
    idx0 = np.argmax(logits, axis=1)
    v0 = logits[np.arange(n), idx0]
    masked = logits.copy()
    masked[np.arange(n), idx0] = -np.inf
    idx1 = np.argmax(masked, axis=1)
    v1 = masked[np.arange(n), idx1]
    m = np.maximum(v0, v1)
    e0 = np.exp(v0 - m)
    e1 = np.exp(v1 - m)
    p0 = e0 / (e0 + e1)
    p1 = e1 / (e0 + e1)
    return np.stack([idx0, idx1], 1), np.stack([p0, p1], 1).astype(np.float32)


def _pack128(a):
    """[R, N] -> [128, (R//128)*N] partition-major layout."""
    R, N = a.shape
    t = R // 128
    return np.ascontiguousarray(
        a.reshape(t, 128, N).transpose(1, 0, 2).reshape(128, t * N))


def kernel(x, gate_w, gate_b, ln1_w, ln1_b, in_proj_w, in_proj_b, out_proj_w,
           out_proj_b, ln2_w, ln2_b, mlp_w1, mlp_b1, mlp_w2, mlp_b2):
    x = np.asarray(x, np.float32)
    B, N, _ = x.shape
    T = B * N
    xf = np.ascontiguousarray(x.reshape(T, E))

    topk_idx, probs = _route(xf, np.asarray(gate_w, np.float32),
                             np.asarray(gate_b, np.float32))

    groups = []          # (token_indices, prob_slice) per core, kk-major
    for kk in range(TOPK):
        for e in range(NE):
            sel = np.nonzero(topk_idx[:, kk] == e)[0]
            groups.append((sel, probs[sel, kk]))
    Cmax = max((len(s) for s, _ in groups), default=128)
    C = max(128, -(-Cmax // 64) * 64)
    KT = -(-C // 128)

    ew = []
    has_vbias = False
    has_b2 = False
    for e in range(NE):
        Wq = np.asarray(in_proj_w[e][0:E], np.float32)
        Wk = np.asarray(in_proj_w[e][E:2 * E], np.float32)
        Wv = np.asarray(in_proj_w[e][2 * E:3 * E], np.float32)
        bq = np.asarray(in_proj_b[e][0:E], np.float32)
        bk = np.asarray(in_proj_b[e][E:2 * E], np.float32)
        bv = np.asarray(in_proj_b[e][2 * E:3 * E], np.float32)
        l1b = np.asarray(ln1_b[e], np.float32)
        l2b = np.asarray(ln2_b[e], np.float32)
        scale = np.float32(1.0) / np.sqrt(np.float32(D))
        wqk = np.concatenate([Wq.T, Wk.T], axis=1) * WS          # [E, 2E]
        bqk = np.concatenate([(Wq @ l1b + bq) * scale, Wk @ l1b + bk])
        vb = (Wv @ l1b + bv) * WS
        w1 = np.asarray(mlp_w1[e], np.float32)
        b2v = np.asarray(mlp_b2[e], np.float32)
        if np.any(vb != 0):
            has_vbias = True
        if np.any(b2v != 0):
            has_b2 = True
        ew.append(dict(
            wqk=_pack128(wqk.astype(np8)),
            bqk=np.ascontiguousarray(bqk, np.float32),
            wv=_pack128((Wv.T * WS).astype(np8)),
            wvb=np.ascontiguousarray(vb.astype(np8)).reshape(1, E),
            wo=_pack128((np.asarray(out_proj_w[e], np.float32).T * WS)
                        .astype(np8)),
            w1=_pack128((w1.T * WS).astype(np8)),
            b1=np.ascontiguousarray(w1 @ l2b + np.asarray(mlp_b1[e],
                                                          np.float32)),
            w2=_pack128((np.asarray(mlp_w2[e], np.float32).T * WS)
                        .astype(np8)),
            b2=b2v,
            bo=np.asarray(out_proj_b[e], np.float32),
            l1w=np.ascontiguousarray(ln1_w[e], np.float32),
            l2w=np.ascontiguousarray(ln2_w[e], np.float32),
        ))

    def colpack(vec, ncol):
        a = np.zeros((128, ncol), np.float32)
        a[:, :] = np.asarray(vec, np.float32).reshape(ncol, 128).T
        return a

    sab = np.zeros((128, 2, 128), np.float32)
    sab[0, 0, 0:64] = 1.0
    sab[32, 0, 64:128] = 1.0
    sab[64, 1, 0:64] = 1.0
    sab[96, 1, 64:128] = 1.0
    selp_np = np.zeros((128, 385), np.float32)
    selp_np[:, 0:256] = sab.reshape(128, 256)
    selp_np[:, 256:384] = 1.0
    selp_np[:, 384] = 1.0 / E

    in_maps = []
    for ci, (sel, _p) in enumerate(groups):
        e = ci % NE
        w = ew[e]
        S = len(sel)
        xg = xf[sel]
        mu_h = xg.mean(1, keepdims=True)
        var_h = ((xg - mu_h) ** 2).mean(1, keepdims=True)
        hg = ((xg - mu_h) / np.sqrt(var_h + EPS) * w["l1w"][None, :])
        hT_np = np.zeros((E, C), np.float32)
        hT_np[:, :S] = hg.T
        xTb_np = np.zeros((E, C), np.float32)
        xTb_np[:, :S] = (xg + w["bo"][None, :]).T
        kbv = np.full((KT * 128,), KEY_PAD_BIAS, np.float32)
        kbv[:max(S, 1)] = 0.0
        consts = np.concatenate([
            colpack(kbv, KT), colpack(w["bqk"], 8), colpack(w["b1"], 16),
            colpack(w["l2w"], 4), colpack(w["b2"], 4)], axis=1)
        im = {"consts": consts, "selp": selp_np,
              "hT": _pack128(hT_np.astype(np8)),
              "xTb": _pack128(xTb_np),
              "wqk": w["wqk"], "wv": w["wv"], "wo": w["wo"],
              "w1": w["w1"], "w2": w["w2"]}
        if has_vbias:
            im["wvb"] = w["wvb"]
        in_maps.append(im)

    nc = _get_prog((C, has_vbias, has_b2))
    res = bass_utils.run_bass_kernel_spmd(
        nc, in_maps, core_ids=list(range(8)),
        trace=bool(int(os.environ.get("KERNEL_TRACE", "0"))))
    kernel.last_exec_time_ns = res.exec_time_ns
    kernel.last_results = res

    out = np.zeros((T, E), np.float32)
    for ci, (sel, p) in enumerate(groups):
        S = len(sel)
        if S == 0:
            continue
        yT = np.asarray(res.results[ci]["yT"], np.float32) \
            .reshape(128, 4, C).transpose(1, 0, 2).reshape(E, C)
        out[sel] += yT[:, :S].T * p[:, None]
    return out.reshape(B, N, E)
